# revision 17
# baseline (speedup 1.0000x reference)
"""Sparse cross-attention kernel for Trainium2 (8 NeuronCores, SPMD).

Problem: nn_CrossAttn (NP=1024 queries, MNP=4096 keys, BS=4, DIM=512,
NH=8 heads, dh=64, TOPK=32 sparse mask shared across heads).

Sharding: core = (batch b, head-group hg).  4 batches x 2 head-groups of 4
heads each.  Each core computes its batch's attention for its 4 heads and
writes per-(head, qt) numerator/denominator blocks; the host divides and
reassembles.

Device algorithm (per core), all matmuls bf16 with f32 PSUM accumulation:
  qhT = (W1h/8) @ qT + b1/8          (ch on partitions, queries free)
  khT = W2h @ kT + b2                (ch on partitions, keys free)
  vh  = vT.T @ W3hT + b3             (keys on partitions, ch free, + ones col)
  per unit u = (phase, kc) with phase = (qt, hp) and kc a 128-key chunk:
    S^T = Kh @ QhT                   two heads of the pair, C=64 each
    numer = exp(S^T)                 ScalarE, PSUM->SBUF bf16
    numerm = numer * maskT[kc]       DVE (h0) / GpSimd (h1, even kc)
    OT  += vh[kc].T @ numerm         65-row col block (64 ch + ones row = den)
  out = OT copied to SBUF, DMA'd out; host divides num/den.

Scheduling: single software-pipelined stream over 128 units with S emitted
2 units ahead of OT; K/V/mask DMAs and projections interleaved in
consumption order; pair-1 K-projection deferred to phase 2 (kt kept
resident); filler matmuls in the ScalarE-bound phases keep the PE p-state
ramped.
"""

import numpy as np
import ml_dtypes

import concourse.bass as bass
import concourse.mybir as mybir
import concourse.tile as tile
from concourse.bass_utils import run_bass_kernel_spmd

BF16 = mybir.dt.bfloat16
F32 = mybir.dt.float32
AF = mybir.ActivationFunctionType
ALU = mybir.AluOpType

NH = 8
DIM = 512
NP = 1024
MNP = 4096
BS = 4
DH = 64
N_CORES = 8
HG_CH = 256          # channels per head-group (4 heads x 64)
NKC = MNP // 128     # 32 key chunks
NQT = 2              # query tiles of 512
PHASES = ((0, 0), (0, 1), (1, 0), (1, 1))   # (qt, hp)

# options test.py can flip
run_opts = {"trace": False, "trace_kwargs": {}}
# tuning knobs
tune = {
    "gps_mask": True,     # GpSimd takes head-1 mask mult on even kc
    "fillers_from": 64,   # unit index from which PE filler matmuls run
    "filler_f": 384,      # filler matmul free-dim size
}
_last_results = {}


def _split_multi_waits(nc):
    """This container's walrus encodes only ONE sync-wait per TPB instruction
    (newer Tile emits several).  Split extras onto preceding NOPs."""
    eng_ok = {
        mybir.EngineType.PE,
        mybir.EngineType.Activation,
        mybir.EngineType.DVE,
        mybir.EngineType.Pool,
        mybir.EngineType.SP,
    }
    for fn in nc.m.functions:
        for blk in fn.blocks:
            insts = blk.instructions
            out = []
            changed = False
            for inst in insts:
                si = inst.sync_info
                if (
                    si is not None
                    and si.on_wait
                    and len(si.on_wait) > 1
                    and inst.engine in eng_ok
                ):
                    waits = list(si.on_wait)
                    for j, w in enumerate(waits[:-1]):
                        out.append(
                            mybir.InstNoOp(
                                name=f"{inst.name}-w{j}",
                                engine=inst.engine,
                                ins=[],
                                outs=[],
                                sync_info=mybir.SyncInfo(on_wait=[w], on_update=[]),
                            )
                        )
                    inst.sync_info = mybir.SyncInfo(
                        on_wait=[waits[-1]], on_update=list(si.on_update)
                    )
                    changed = True
                out.append(inst)
            if changed:
                blk.instructions = out


def _build_nc() -> bass.Bass:
    nc = bass.Bass()

    qt_d = nc.dram_tensor("qt", [2, 128, 4, 512], BF16, kind="ExternalInput")
    kt_d = nc.dram_tensor("kt", [8, 128, 4, 512], BF16, kind="ExternalInput")
    vt_d = nc.dram_tensor("vt", [NKC, 128, 4, 128], BF16, kind="ExternalInput")
    w1t_d = nc.dram_tensor("w1t", [128, 4, HG_CH], BF16, kind="ExternalInput")
    w2t_d = nc.dram_tensor("w2t", [128, 4, HG_CH], BF16, kind="ExternalInput")
    w3t_d = nc.dram_tensor("w3t", [128, 4, 260], BF16, kind="ExternalInput")
    b1c_d = nc.dram_tensor("b1c", [128, 2], F32, kind="ExternalInput")
    b2c_d = nc.dram_tensor("b2c", [128, 2], F32, kind="ExternalInput")
    b3r_d = nc.dram_tensor("b3r", [1, 260], BF16, kind="ExternalInput")
    mask_d = nc.dram_tensor("maskt", [128, NQT, NKC, 512], BF16, kind="ExternalInput")
    ones128_d = nc.dram_tensor("ones128", [1, 128], BF16, kind="ExternalInput")
    out_d = nc.dram_tensor("outt", [2, 2, 65, NQT, 512], F32, kind="ExternalOutput")

    with tile.TileContext(nc) as tc:
        with (
            tc.tile_pool(name="const", bufs=1) as const,
            tc.tile_pool(name="big", bufs=1) as big,
            tc.tile_pool(name="vio", bufs=5) as vio,
            tc.tile_pool(name="work", bufs=4) as work,
            tc.tile_pool(name="fin", bufs=2) as fin,
            tc.tile_pool(name="ps_s", bufs=2, space="PSUM") as ps_s,
            tc.tile_pool(name="ps_o", bufs=1, space="PSUM") as ps_o,
            tc.tile_pool(name="ps_p", bufs=2, space="PSUM") as ps_p,
        ):
            # ---- constants / weights ----
            w1t = const.tile([128, 4, HG_CH], BF16)
            w2t = const.tile([128, 4, HG_CH], BF16)
            w3t = const.tile([128, 4, 260], BF16)
            b1c = const.tile([128, 2], F32)
            b2c = const.tile([128, 2], F32)
            b3r = const.tile([1, 260], BF16)
            ones128 = const.tile([1, 128], BF16)

            # ---- big resident tensors ----
            qts = big.tile([128, 4, NP], BF16)
            kts = big.tile([128, 4, MNP], BF16)
            maskt = big.tile([128, NQT, NKC, 512], BF16)
            qhT = big.tile([128, 2, NP], BF16)   # [ch_in_pair, pair, query]
            khT = big.tile([128, 2, MNP], BF16)  # [ch_in_pair, pair, key]
            vh = big.tile([128, NKC, 260], BF16)  # [key_in_chunk, kc, 4*(64ch+1)]

            # first DMAs: what the prologue needs, in consumption order.
            # Cold-start consts go via the idle DVE queue so the SP queue
            # reaches the k chunk 0 DMA as early as possible.
            nc.sync.dma_start(out=qts[:, :, 0:512], in_=qt_d[0])
            nc.sync.dma_start(out=w1t[:], in_=w1t_d[:])
            nc.scalar.dma_start(out=b1c[:], in_=b1c_d[:])
            nc.scalar.dma_start(out=b2c[:], in_=b2c_d[:])
            nc.scalar.dma_start(out=w3t[:], in_=w3t_d[:])
            nc.scalar.dma_start(out=b3r[:], in_=b3r_d[:])
            nc.scalar.dma_start(out=ones128[:], in_=ones128_d[:])
            nc.sync.dma_start(out=kts[:, :, 0:512], in_=kt_d[0])
            nc.sync.dma_start(out=w2t[:], in_=w2t_d[:])

            # preload the exp activation table while DMAs stream
            pre = work.tile([128, 2], BF16, tag="pre", bufs=1, name="pre")
            nc.scalar.activation(pre[:], b1c[:], AF.Exp)

            # ---- helpers ----
            def dma_kt(nq):
                nc.sync.dma_start(
                    out=kts[:, :, nq * 512:(nq + 1) * 512],
                    in_=kt_d[nq],
                )

            def dma_mask(qt, c):
                nc.sync.dma_start(
                    out=maskt[:, qt, 4 * c:4 * (c + 1), :],
                    in_=mask_d[:, qt, 4 * c:4 * (c + 1), :],
                )

            vts_tiles = {}

            def dma_v(kc):
                t = vio.tile([128, 4, 128], BF16, tag="vts", name=f"vts{kc}")
                nc.sync.dma_start(out=t[:], in_=vt_d[kc])
                vts_tiles[kc] = t

            def proj_q(pair, nq2):
                pt = ps_p.tile([128, 512], F32, tag="pp", name=f"qpp{pair}{nq2}")
                for c in range(4):
                    nc.tensor.matmul(
                        pt[:],
                        lhsT=w1t[:, c, pair * 128:(pair + 1) * 128],
                        rhs=qts[:, c, nq2 * 512:(nq2 + 1) * 512],
                        start=(c == 0),
                        stop=(c == 3),
                    )
                nc.vector.tensor_tensor(
                    out=qhT[:, pair, nq2 * 512:(nq2 + 1) * 512],
                    in0=pt[:],
                    in1=b1c[:, pair:pair + 1].to_broadcast((128, 512)),
                    op=ALU.add,
                )

            def proj_k(nq, pair):
                pt = ps_p.tile([128, 512], F32, tag="pp", name=f"kpp{pair}{nq}")
                for c in range(4):
                    nc.tensor.matmul(
                        pt[:],
                        lhsT=w2t[:, c, pair * 128:(pair + 1) * 128],
                        rhs=kts[:, c, nq * 512:(nq + 1) * 512],
                        start=(c == 0),
                        stop=(c == 3),
                    )
                nc.vector.tensor_tensor(
                    out=khT[:, pair, nq * 512:(nq + 1) * 512],
                    in0=pt[:],
                    in1=b2c[:, pair:pair + 1].to_broadcast((128, 512)),
                    op=ALU.add,
                )

            # vh projection split into single matmuls so they can be
            # interleaved between the 512-col S/OT matmuls: the ~100ns
            # weight load of each small F=260 matmul then hides under a
            # long-running neighbor instead of serializing on the PE.
            vpt_tiles = {}

            def proj_v_mm(kc, j):
                if j == 0:
                    vpt_tiles[kc] = ps_p.tile(
                        [128, 260], F32, tag="pp", name=f"vpp{kc}"
                    )
                pt = vpt_tiles[kc]
                if j < 4:
                    nc.tensor.matmul(
                        pt[:],
                        lhsT=vts_tiles[kc][:, j, :],
                        rhs=w3t[:, j, :],
                        start=(j == 0),
                        stop=False,
                    )
                else:
                    nc.tensor.matmul(
                        pt[:], lhsT=ones128[:], rhs=b3r[:], start=False, stop=True,
                    )

            def proj_v_copy(kc):
                del vts_tiles[kc]
                pt = vpt_tiles.pop(kc)
                # PSUM->SBUF downcast on ScalarE: it idles during the
                # PE-bound phase 1 while the DVE is near-saturated there.
                nc.scalar.copy(out=vh[:, kc, :], in_=pt[:])

            def proj_v(kc):
                for j in range(5):
                    proj_v_mm(kc, j)
                proj_v_copy(kc)

            # ---- attention unit machinery ----
            numerm_by_u = {}
            o_ps_by_p = {}

            s_ps_by_u = {}

            def S_mm(u, h):
                p, kc = u // 32, u % 32
                qt, hp = PHASES[p]
                if h == 0:
                    s_ps_by_u[u] = ps_s.tile(
                        [128, 1024], F32, tag="s", name=f"s{u}"
                    )
                s_ps = s_ps_by_u[u]
                nc.tensor.matmul(
                    s_ps[:, h * 512:(h + 1) * 512],
                    lhsT=khT[h * 64:(h + 1) * 64, hp, kc * 128:(kc + 1) * 128],
                    rhs=qhT[h * 64:(h + 1) * 64, hp, qt * 512:(qt + 1) * 512],
                    start=True,
                    stop=True,
                )

            def exp_mask(u):
                p, kc = u // 32, u % 32
                qt, hp = PHASES[p]
                s_ps = s_ps_by_u.pop(u)
                numer = work.tile([128, 1024], BF16, tag="numer", name=f"nu{u}")
                nc.scalar.activation(numer[:], s_ps[:], AF.Exp)
                numerm = work.tile([128, 1024], BF16, tag="numerm", name=f"nm{u}")
                for h in range(2):
                    eng = nc.vector
                    if tune["gps_mask"] and h == 1 and kc % 2 == 0 and u < 64:
                        eng = nc.gpsimd
                    eng.tensor_tensor(
                        out=numerm[:, h * 512:(h + 1) * 512],
                        in0=numer[:, h * 512:(h + 1) * 512],
                        in1=maskt[:, qt, kc, :],
                        op=ALU.mult,
                    )
                numerm_by_u[u] = numerm

            def emit_S(u):
                S_mm(u, 0)
                S_mm(u, 1)
                exp_mask(u)

            def OT_mm(u, h):
                p, kc = u // 32, u % 32
                qt, hp = PHASES[p]
                if kc == 0 and h == 0:
                    o_ps_by_p[p] = [
                        ps_o.tile([65, 512], F32, tag=f"o{hh}", name=f"ops{p}{hh}")
                        for hh in range(2)
                    ]
                o_ps = o_ps_by_p[p]
                numerm = numerm_by_u[u]
                ch = (2 * hp + h) * 65
                nc.tensor.matmul(
                    o_ps[h][:],
                    lhsT=vh[:, kc, ch:ch + 65],
                    rhs=numerm[:, h * 512:(h + 1) * 512],
                    start=(kc == 0),
                    stop=(kc == NKC - 1),
                )
                if h == 1:
                    del numerm_by_u[u]

            def emit_OT(u):
                OT_mm(u, 0)
                OT_mm(u, 1)

            def tail(p):
                qt, hp = PHASES[p]
                o_ps = o_ps_by_p.pop(p)
                for h in range(2):
                    osb = fin.tile([65, 512], F32, tag=f"osb{h}", name=f"osb{p}{h}")
                    nc.vector.tensor_copy(out=osb[:], in_=o_ps[h][:])
                    nc.sync.dma_start(out=out_d[hp, h, :, qt, :], in_=osb[:])

            fill_n = [0]

            def filler():
                fill_n[0] += 1
                fp = ps_p.tile([128, 512], F32, tag="pp", name=f"fill{fill_n[0]}")
                nc.tensor.matmul(
                    fp[:, 0:tune["filler_f"]],
                    lhsT=w2t[:, 0, 0:128],
                    rhs=qts[:, 0, 0:tune["filler_f"]],
                    start=True,
                    stop=True,
                    skip_group_check=True,
                )

            def unit_phase1(u):
                """Interleave the small vh-projection matmuls between the
                512-col S/OT matmuls so their weight loads pipeline."""
                nq = u // 4
                if u % 4 == 0 and nq + 1 <= 7:
                    dma_kt(nq + 1)
                if u % 4 == 1 and nq + 1 <= 7:
                    dma_mask(0, nq + 1)
                if u % 4 == 2 and nq + 1 <= 7:
                    proj_k(nq + 1, 0)
                if u + 4 <= 31:
                    dma_v(u + 4)
                if u == 2:
                    nc.sync.dma_start(out=qts[:, :, 512:1024], in_=qt_d[1])
                vkc = u + 2 if u + 2 <= 31 else None
                su = u + 2
                S_mm(su, 0)
                if vkc is not None:
                    proj_v_mm(vkc, 0)
                S_mm(su, 1)
                if vkc is not None:
                    proj_v_mm(vkc, 1)
                exp_mask(su)
                if u == 8:
                    proj_q(0, 1)
                if u == 25:
                    proj_q(1, 0)
                if u == 26:
                    proj_k(0, 1)
                if u == 27:
                    proj_q(1, 1)
                OT_mm(u, 0)
                if vkc is not None:
                    proj_v_mm(vkc, 2)
                OT_mm(u, 1)
                if vkc is not None:
                    proj_v_mm(vkc, 3)
                    proj_v_mm(vkc, 4)
                    proj_v_copy(vkc)

            def unit_rest(u):
                if u < 64:
                    kc = u - 32
                    if kc % 4 == 2 and kc // 4 + 1 <= 7:
                        proj_k(kc // 4 + 1, 1)
                    if kc % 4 == 1 and kc // 4 <= 7:
                        dma_mask(1, kc // 4)
                if u + 2 <= 127:
                    emit_S(u + 2)
                emit_OT(u)
                if u >= tune["fillers_from"]:
                    filler()

            # ---- prologue ----
            proj_q(0, 0)
            dma_mask(0, 0)
            proj_k(0, 0)
            for kc in range(4):
                dma_v(kc)
            emit_S(0)
            emit_S(1)
            proj_v(0)
            proj_v(1)

            # ---- main pipelined stream ----
            for u in range(128):
                if u < 32:
                    unit_phase1(u)
                else:
                    unit_rest(u)
                if u % 32 == 31:
                    tail(u // 32)

    _split_multi_waits(nc)
    return nc


def _prep_inputs(q, k, v, rns_indices, W1, b1, W2, b2, W3, b3):
    bf = ml_dtypes.bfloat16
    q = np.asarray(q, np.float32)
    k = np.asarray(k, np.float32)
    v = np.asarray(v, np.float32)
    idx = np.asarray(rns_indices)
    W1 = np.asarray(W1, np.float32)
    W2 = np.asarray(W2, np.float32)
    W3 = np.asarray(W3, np.float32)
    b1 = np.asarray(b1, np.float32)
    b2 = np.asarray(b2, np.float32)
    b3 = np.asarray(b3, np.float32)
    scale = 1.0 / np.sqrt(DH)

    def part3(x2d, n):  # (512, n) -> (128, 4, n)
        return np.ascontiguousarray(
            x2d.reshape(4, 128, n).transpose(1, 0, 2)
        ).astype(bf)

    def _aug_w3(W3h):  # (256, 512) -> (128, 4, 260) with zero cols at ones slots
        wt = np.zeros((DIM, 260), np.float32)
        for h in range(4):
            wt[:, h * 65:h * 65 + 64] = W3h[h * 64:(h + 1) * 64, :].T
        return part3(wt, 260)

    def _aug_b3(b3h):  # (256,) -> (1, 260) with 1.0 at ones slots
        br = np.zeros((1, 260), np.float32)
        for h in range(4):
            br[0, h * 65:h * 65 + 64] = b3h[h * 64:(h + 1) * 64]
            br[0, h * 65 + 64] = 1.0
        return br.astype(bf)

    masks = []
    for b in range(BS):
        m = np.zeros((NP, MNP), np.float32)
        m[np.arange(NP)[:, None], idx[b]] = 1.0
        # [keys, queries] -> [128 key-in-chunk, 2 qt, 32 kc, 512 q]
        mt = m.T.reshape(NKC, 128, NQT, 512).transpose(1, 2, 0, 3)
        masks.append(np.ascontiguousarray(mt).astype(bf))

    def chunk_major(arr, n):  # (128, 4, N) -> (N//n, 128, 4, n)
        nchunks = arr.shape[2] // n
        return np.ascontiguousarray(
            arr.reshape(128, 4, nchunks, n).transpose(2, 0, 1, 3)
        )

    qkv_t = []
    for b in range(BS):
        qkv_t.append(
            (
                chunk_major(part3(q[:, b, :].T, NP), 512),
                chunk_major(part3(k[:, b, :].T, MNP), 512),
                chunk_major(part3(v[:, b, :].T, MNP), 128),
            )
        )

    in_maps = []
    for core in range(N_CORES):
        b, hg = core // 2, core % 2
        sl = slice(hg * HG_CH, (hg + 1) * HG_CH)
        qtb, ktb, vtb = qkv_t[b]
        im = {
            "qt": qtb,
            "kt": ktb,
            "vt": vtb,
            "w1t": part3(W1[sl, :].T * scale, HG_CH),
            "w2t": part3(W2[sl, :].T, HG_CH),
            "w3t": _aug_w3(W3[sl, :]),
            "b1c": np.ascontiguousarray(
                (b1[sl] * scale).reshape(2, 128).T
            ).astype(np.float32),
            "b2c": np.ascontiguousarray(b2[sl].reshape(2, 128).T).astype(np.float32),
            "b3r": _aug_b3(b3[sl]),
            "maskt": masks[b],
            "ones128": np.ones((1, 128), bf),
        }
        in_maps.append(im)
    return in_maps


def kernel(q, k, v, rns_indices, W1, b1, W2, b2, W3, b3):
    nc = _build_nc()
    in_maps = _prep_inputs(q, k, v, rns_indices, W1, b1, W2, b2, W3, b3)
    res = run_bass_kernel_spmd(
        nc,
        in_maps,
        core_ids=list(range(N_CORES)),
        trace=run_opts["trace"],
        **run_opts["trace_kwargs"],
    )
    _last_results["res"] = res

    out = np.empty((NP, BS, DIM), np.float32)
    for core in range(N_CORES):
        b, hg = core // 2, core % 2
        r = np.asarray(res.results[core]["outt"], np.float32)  # (2,2,65,2,512)
        for hp in range(2):
            for h in range(2):
                num = r[hp, h, 0:64, :, :]          # (64, 2, 512)
                den = r[hp, h, 64, :, :]            # (2, 512)
                blk = (num / den[None]).transpose(1, 2, 0).reshape(NP, 64)
                ch0 = hg * HG_CH + (2 * hp + h) * 64
                out[:, b, ch0:ch0 + 64] = blk
    return out
